# revision 12
# baseline (speedup 1.0000x reference)
"""CWCFace head (nn_CWCFace_11201274708637) — Trainium2 Bass kernel.

Math (reference):
    kn = kernel / ||kernel||_col
    cos = clip(emb @ kn, -1+eps, 1-eps)              # [B, C]
    ms  = margin_scaler(norms, label)                # [B, 1] per-sample stats
    th  = arccos(cos); th_m = clip(th + onehot*(-M*ms), eps, pi-eps)
    out = (cos(th_m) - onehot*(M + M*ms)) * S

The onehot terms touch exactly ONE column per row, so the full [B, C]
tensor only needs  out = clip(S*cos)  plus a B-element fix-up at
(i, label_i).  cos(th+g) for those B elements uses the identity
cos(th+g) = t*cos(g) - sqrt(1-t^2)*sin(g); the theta-clip branches are
threshold comparisons — no arccos.

Device kernel = one big bf16 matmul + clamp + scatter fix-up:
  - the column normalization and the S scale are folded into the bf16
    kernel upload (host-side input prep), so psum is S*cos directly
  - epilogue is a single DVE tensor_scalar clamp PSUM(f32) -> SBUF bf16,
    output DMA is bf16 (half the write traffic of f32)
  - fix-up values come from per-sample dot products against the label's
    kernel column (host gathers the columns; device does the math), so
    no gather-after-store serial tail — just 4 indirect scatters that
    wait on the stores of their B-tile
  - margin stats (segment count/sum/sumsq) via BxB label-equality
    matmul, as before

Sharding: classes column-split over 8 cores, CS=8848 per core
(8*8848 = 70784 >= 70722).  Kernel blocks are uploaded pre-swizzled to
[128, KT*W] so every block load is one long contiguous DMA per
partition; blocks are prefetched 2 ahead to keep the PE gap-free (and
at the max p-state).
"""

import sys

for _p in (
    "/root/.axon_site",
    "/root/.axon_site/_ro/trn_rl_repo",
    "/root/.axon_site/_ro/pypackages",
    "/opt/trn_rl_repo",
):
    if _p not in sys.path:
        sys.path.append(_p)

import math

import numpy as np

import concourse.bass as bass
import concourse.mybir as mybir
import concourse.tile as tile
from concourse import bacc
from concourse.bass import IndirectOffsetOnAxis
from concourse.bass_utils import run_bass_kernel_spmd

B = 512
EMB = 512
C = 70722
NCORES = 8
CS = 8848  # per-core classes (padded);  8 * 8848 = 70784 >= 70722
S = 64.0
MARG = 0.4
H = 0.333
EPS = 1e-3

F32 = mybir.dt.float32
BF16 = mybir.dt.bfloat16
I32 = mybir.dt.int32
AL = mybir.AluOpType
AF = mybir.ActivationFunctionType

KT = EMB // 128          # 4 K-tiles
BT = B // 128            # 4 B-tiles
COS_EPS = float(math.cos(EPS))
PI_2 = math.pi / 2.0
CLAMP = S * (1.0 - EPS)


def _blocks():
    """Kernel-load blocks: (c0, Wb, [(c0_sub, W), ...]).

    First two blocks are single 512-wide slices so the PE can start
    early; the rest pair two slices per load (1MB transfers)."""
    slices = []
    c0 = 0
    while c0 < CS:
        w = min(512, CS - c0)
        slices.append((c0, w))
        c0 += w
    blocks = []
    bi = 0
    while bi < len(slices):
        group = slices[bi : bi + 1] if len(blocks) < 2 else slices[bi : bi + 2]
        blocks.append((group[0][0], sum(w for _, w in group), group))
        bi += len(group)
    return blocks


def _emit(nc, tc, embT_h, kern_h, labf_h, nrm_h, labrow_h,
          embrow_h, kncol_h, fix_h, out_hs):
    out2ds = [
        oh[:, :].rearrange("(p c) o -> p (c o)", c=CS) for oh in out_hs
    ]  # [128, CS] each

    cst_cm = tc.tile_pool(name="cst", bufs=1)
    cst = cst_cm.__enter__()

    embT_sb = cst.tile([128, KT, B], BF16, tag="embT")      # [p, k, b]
    embrow_sb = cst.tile([128, BT, EMB], BF16, tag="embrow")
    kncol_sb = cst.tile([128, BT, EMB], BF16, tag="kncol")
    labf_sb = cst.tile([128, BT], F32, tag="labf")
    nrm_sb = cst.tile([128, BT], F32, tag="nrm")
    labrow_sb = cst.tile([1, B], F32, tag="labrow")
    g_sb = cst.tile([128, BT], F32, tag="g")        # -M * ms
    gadd_sb = cst.tile([128, BT], F32, tag="gadd")  # M + M * ms
    v_sb = cst.tile([128, BT], F32, tag="v")        # safe norms

    def load_embT():
        # split by k so the very first matmul can start after ~160KB
        for k in range(KT):
            nc.sync.dma_start(
                out=embT_sb[:, k, :], in_=embT_h[:, k * B : (k + 1) * B]
            )

    def load_small():
        nc.sync.dma_start(out=embrow_sb[:], in_=embrow_h[:, :].rearrange(
            "p (t e) -> p t e", t=BT))
        nc.sync.dma_start(out=kncol_sb[:], in_=kncol_h[:, :].rearrange(
            "p (t e) -> p t e", t=BT))
        nc.sync.dma_start(out=labf_sb[:], in_=labf_h[:, :])
        nc.sync.dma_start(out=nrm_sb[:], in_=nrm_h[:, :])
        nc.sync.dma_start(out=labrow_sb[:], in_=labrow_h[:, :])

    blocks = _blocks()
    kernR = kern_h  # [128, KT*CS], block bi at columns KT*c0 .. KT*(c0+Wb)

    with (
        tc.tile_pool(name="pa", bufs=2) as pa,
        tc.tile_pool(name="kp", bufs=5) as kp,
        tc.tile_pool(name="op", bufs=10) as op_,
        tc.tile_pool(name="ps", bufs=8, space="PSUM") as ps,
        tc.tile_pool(name="pc", bufs=1) as pc,
    ):
        store_insts = [[] for _ in range(BT)]

        def load_block(bidx, split_k=False):
            c0b, Wb, _ = blocks[bidx]
            ksb = kp.tile([128, KT, Wb], BF16, tag="ks")
            src = kernR[:, KT * c0b : KT * (c0b + Wb)].rearrange(
                "p (k w) -> p k w", k=KT
            )
            if split_k:
                for k in range(KT):
                    nc.sync.dma_start(out=ksb[:, k, :], in_=src[:, k, :])
            else:
                nc.sync.dma_start(out=ksb[:], in_=src)
            return ksb

        def main_slice(c0, W, ksb, off):
            for b in range(BT):
                ps_out = ps.tile([128, W], F32, space="PSUM", tag="po")
                for k in range(KT):
                    nc.tensor.matmul(
                        ps_out[:],
                        embT_sb[:, k, b * 128 : (b + 1) * 128],
                        ksb[:, k, off : off + W],
                        start=(k == 0),
                        stop=(k == KT - 1),
                    )
                o_sb = op_.tile([128, W], BF16, tag="o")
                nc.vector.tensor_scalar(
                    o_sb[:], ps_out[:], -CLAMP, CLAMP, op0=AL.max, op1=AL.min
                )
                st = nc.sync.dma_start(
                    out=out2ds[b][:, c0 : c0 + W], in_=o_sb[:]
                )
                store_insts[b].append(st.ins)

        def phase_a():
            """Margin-scaler segment stats -> g_sb, gadd_sb."""
            labAll = pa.tile([128, B], F32, tag="labAll")
            nc.gpsimd.partition_broadcast(labAll[:], labrow_sb[:])

            nc.vector.tensor_scalar(
                v_sb[:], nrm_sb[:], 0.001, 100.0, op0=AL.max, op1=AL.min
            )
            w_sb = pa.tile([128, 3 * BT], F32, tag="w")
            nc.vector.memset(w_sb[:], 1.0)
            for b in range(BT):
                nc.vector.tensor_copy(
                    w_sb[:, 3 * b + 1 : 3 * b + 2], v_sb[:, b : b + 1]
                )
                nc.vector.tensor_tensor(
                    out=w_sb[:, 3 * b + 2 : 3 * b + 3],
                    in0=v_sb[:, b : b + 1],
                    in1=v_sb[:, b : b + 1],
                    op=AL.mult,
                )

            st_all = pa.tile([128, 3 * BT], F32, tag="st_all")
            for a in range(BT):
                ps_st = ps.tile([128, 3], F32, space="PSUM", tag="po")
                for b in range(BT):
                    eq = pa.tile([128, 128], F32, tag="eq")
                    nc.vector.tensor_tensor(
                        out=eq[:],
                        in0=labf_sb[:, b : b + 1].to_broadcast([128, 128]),
                        in1=labAll[:, a * 128 : (a + 1) * 128],
                        op=AL.is_equal,
                    )
                    nc.tensor.matmul(
                        ps_st[:],
                        eq[:],
                        w_sb[:, 3 * b : 3 * b + 3],
                        start=(b == 0),
                        stop=(b == BT - 1),
                    )
                nc.vector.tensor_copy(st_all[:, 3 * a : 3 * a + 3], ps_st[:])

            stv = st_all[:].rearrange("p (a c) -> p a c", c=3)
            n_ = stv[:, :, 0]
            sm = stv[:, :, 1]
            sq2 = stv[:, :, 2]

            t0 = pa.tile([128, 8 * BT], F32, tag="t0")
            tv = t0[:].rearrange("p (i a) -> p i a", a=BT)
            rn = tv[:, 0, :]
            nc.vector.reciprocal(rn, n_)
            mean = tv[:, 1, :]
            nc.vector.tensor_tensor(out=mean, in0=sm, in1=rn, op=AL.mult)
            m2 = tv[:, 2, :]
            nc.vector.tensor_tensor(out=m2, in0=mean, in1=mean, op=AL.mult)
            nm2 = tv[:, 3, :]
            nc.vector.tensor_tensor(out=nm2, in0=n_, in1=m2, op=AL.mult)
            num = tv[:, 4, :]
            nc.vector.tensor_tensor(out=num, in0=sq2, in1=nm2, op=AL.subtract)
            den = tv[:, 5, :]
            nc.vector.tensor_scalar(den, n_, -1.0, 1.0, op0=AL.add, op1=AL.max)
            rden = tv[:, 6, :]
            nc.vector.reciprocal(rden, den)
            var = tv[:, 7, :]
            nc.vector.tensor_tensor(out=var, in0=num, in1=rden, op=AL.mult)
            nc.vector.tensor_scalar(var, var, 1e-30, None, op0=AL.max)

            t1 = pa.tile([128, 8 * BT], F32, tag="t1")
            uv = t1[:].rearrange("p (i a) -> p i a", a=BT)
            ars = uv[:, 0, :]
            nc.scalar.activation(ars, var, AF.Abs_reciprocal_sqrt)
            std = uv[:, 1, :]
            nc.vector.tensor_tensor(out=std, in0=var, in1=ars, op=AL.mult)
            stdp = uv[:, 2, :]
            nc.vector.tensor_scalar(stdp, std, EPS, None, op0=AL.add)
            rstd = uv[:, 3, :]
            nc.vector.reciprocal(rstd, stdp)
            mask = uv[:, 4, :]
            nc.vector.tensor_scalar(mask, n_, 2.0, None, op0=AL.is_gt)
            mask_i = pa.tile([128, BT], I32, tag="mask_i")
            nc.vector.tensor_copy(mask_i[:], mask)
            c05 = uv[:, 5, :]
            nc.vector.memset(c05, 0.05)
            invd = uv[:, 6, :]
            nc.vector.select(invd, mask_i[:], rstd, c05)
            dv = uv[:, 7, :]
            nc.vector.tensor_tensor(out=dv, in0=v_sb[:], in1=mean, op=AL.subtract)
            res = tv[:, 0, :]
            nc.vector.tensor_tensor(out=res, in0=dv, in1=invd, op=AL.mult)
            ms = tv[:, 1, :]
            nc.vector.tensor_scalar(ms, res, H, 1.0, op0=AL.mult, op1=AL.min)
            nc.vector.tensor_scalar(ms, ms, -1.0, None, op0=AL.max)
            nc.vector.tensor_scalar(g_sb[:], ms, -MARG, None, op0=AL.mult)
            nc.vector.tensor_scalar(
                gadd_sb[:], ms, MARG, MARG, op0=AL.mult, op1=AL.add
            )

        def phase_c_pre():
            """Angle thresholds (needs g_sb)."""
            pcst = {}
            cpi2 = pc.tile([128, 1], F32, tag="cpi2")
            nc.vector.memset(cpi2[:], PI_2)
            cpie = pc.tile([128, 1], F32, tag="cpie")
            nc.vector.memset(cpie[:], PI_2 + EPS)

            cosg = pc.tile([128, BT], F32, tag="cosg")
            sing = pc.tile([128, BT], F32, tag="sing")
            thr_lo = pc.tile([128, BT], F32, tag="thr_lo")
            thr_hi = pc.tile([128, BT], F32, tag="thr_hi")
            for b in range(BT):
                gb = g_sb[:, b : b + 1]
                nc.scalar.activation(cosg[:, b : b + 1], gb, AF.Sin, bias=cpi2[:])
                nc.scalar.activation(sing[:, b : b + 1], gb, AF.Sin)
                nc.scalar.activation(
                    thr_lo[:, b : b + 1], gb, AF.Sin, bias=cpie[:], scale=-1.0
                )
                nc.scalar.activation(
                    thr_hi[:, b : b + 1], gb, AF.Sin, bias=cpie[:], scale=1.0
                )
            nthr = pc.tile([128, BT], F32, tag="nthr")
            nc.vector.tensor_scalar(nthr[:], thr_hi[:], -1.0, None, op0=AL.mult)
            ml1 = pc.tile([128, BT], F32, tag="ml1")
            nc.vector.tensor_scalar(ml1[:], g_sb[:], EPS, None, op0=AL.is_lt)
            mh1 = pc.tile([128, BT], F32, tag="mh1")
            nc.vector.tensor_scalar(mh1[:], g_sb[:], -EPS, None, op0=AL.is_gt)
            c_lo = pc.tile([128, BT], F32, tag="c_lo")
            nc.vector.memset(c_lo[:], COS_EPS)
            c_hi = pc.tile([128, BT], F32, tag="c_hi")
            nc.vector.memset(c_hi[:], -COS_EPS)
            pcst.update(
                cosg=cosg, sing=sing, thr_lo=thr_lo, nthr=nthr,
                ml1=ml1, mh1=mh1, c_lo=c_lo, c_hi=c_hi,
            )
            return pcst

        def phase_c_vals(pcst):
            """Fix-up values from direct dot products (no HBM gather)."""
            traw = pc.tile([128, BT], F32, tag="traw")
            scr = pc.tile([128, EMB], F32, tag="scr")
            for b in range(BT):
                nc.vector.scalar_tensor_tensor(
                    out=scr[:],
                    in0=embrow_sb[:, b, :],
                    scalar=1.0,
                    in1=kncol_sb[:, b, :],
                    op0=AL.mult,
                    op1=AL.mult,
                    accum_out=traw[:, b : b + 1],
                )
            t_ = pc.tile([128, BT], F32, tag="t_")
            nc.vector.tensor_scalar(
                t_[:], traw[:], 1.0 / S, 1.0 - EPS, op0=AL.mult, op1=AL.min
            )
            nc.vector.tensor_scalar(t_[:], t_[:], -1.0 + EPS, None, op0=AL.max)

            t2 = pc.tile([128, BT], F32, tag="t2")
            nc.vector.tensor_tensor(out=t2[:], in0=t_[:], in1=t_[:], op=AL.mult)
            om = pc.tile([128, BT], F32, tag="om")
            nc.vector.tensor_scalar(om[:], t2[:], -1.0, 1.0, op0=AL.mult, op1=AL.add)
            omr = pc.tile([128, BT], F32, tag="omr")
            nc.scalar.activation(omr[:], om[:], AF.Abs_reciprocal_sqrt)
            sq = pc.tile([128, BT], F32, tag="sq")
            nc.vector.tensor_tensor(out=sq[:], in0=om[:], in1=omr[:], op=AL.mult)

            a1 = pc.tile([128, BT], F32, tag="a1")
            nc.vector.tensor_tensor(out=a1[:], in0=t_[:], in1=pcst["cosg"][:], op=AL.mult)
            a2 = pc.tile([128, BT], F32, tag="a2")
            nc.vector.tensor_tensor(out=a2[:], in0=sq[:], in1=pcst["sing"][:], op=AL.mult)
            cosm = pc.tile([128, BT], F32, tag="cosm")
            nc.vector.tensor_tensor(out=cosm[:], in0=a1[:], in1=a2[:], op=AL.subtract)

            ml2 = pc.tile([128, BT], F32, tag="ml2")
            nc.vector.tensor_tensor(
                out=ml2[:], in0=t_[:], in1=pcst["thr_lo"][:], op=AL.is_gt
            )
            mlow = pc.tile([128, BT], F32, tag="mlow")
            nc.vector.tensor_tensor(out=mlow[:], in0=pcst["ml1"][:], in1=ml2[:], op=AL.mult)
            mh2 = pc.tile([128, BT], F32, tag="mh2")
            nc.vector.tensor_tensor(
                out=mh2[:], in0=t_[:], in1=pcst["nthr"][:], op=AL.is_lt
            )
            mhigh = pc.tile([128, BT], F32, tag="mhigh")
            nc.vector.tensor_tensor(out=mhigh[:], in0=pcst["mh1"][:], in1=mh2[:], op=AL.mult)

            mlow_i = pc.tile([128, BT], I32, tag="mlow_i")
            nc.vector.tensor_copy(mlow_i[:], mlow[:])
            mhigh_i = pc.tile([128, BT], I32, tag="mhigh_i")
            nc.vector.tensor_copy(mhigh_i[:], mhigh[:])
            nc.vector.select(cosm[:], mlow_i[:], pcst["c_lo"][:], cosm[:])
            nc.vector.select(cosm[:], mhigh_i[:], pcst["c_hi"][:], cosm[:])

            val = pc.tile([128, BT], F32, tag="val")
            nc.vector.tensor_tensor(
                out=val[:], in0=cosm[:], in1=gadd_sb[:], op=AL.subtract
            )
            nc.vector.tensor_scalar(val[:], val[:], S, None, op0=AL.mult)
            nc.sync.dma_start(out=fix_h[:, :], in_=val[:])

        # ---- emission ----
        load_embT()
        ksbs = {0: load_block(0, split_k=True), 1: load_block(1)}
        load_small()

        all_slices = []  # (c0, W, block_idx, off)
        for bidx, (_, _, group) in enumerate(blocks):
            off = 0
            for c0, W in group:
                all_slices.append((c0, W, bidx, off))
                off += W

        loaded = 2
        pcst = None
        for si, (c0, W, bidx, off) in enumerate(all_slices):
            while loaded < len(blocks) and loaded <= bidx + 2:
                ksbs[loaded] = load_block(loaded)
                loaded += 1
            main_slice(c0, W, ksbs[bidx], off)
            if si == 0:
                phase_a()
                pcst = phase_c_pre()
            elif si == 1:
                phase_c_vals(pcst)

    cst_cm.__exit__(None, None, None)


def _build():
    nc = bacc.Bacc(
        "TRN2", target_bir_lowering=False, debug=False, num_devices=NCORES
    )
    embT_h = nc.dram_tensor("embT", [128, KT * B], BF16, kind="ExternalInput")
    kern_h = nc.dram_tensor("kern", [128, KT * CS], BF16, kind="ExternalInput")
    labf_h = nc.dram_tensor("labf", [128, BT], F32, kind="ExternalInput")
    nrm_h = nc.dram_tensor("nrm", [128, BT], F32, kind="ExternalInput")
    labrow_h = nc.dram_tensor("labrow", [1, B], F32, kind="ExternalInput")
    embrow_h = nc.dram_tensor("embrow", [128, BT * EMB], BF16, kind="ExternalInput")
    kncol_h = nc.dram_tensor("kncol", [128, BT * EMB], BF16, kind="ExternalInput")
    fix_h = nc.dram_tensor("fix", [128, BT], F32, kind="ExternalOutput")
    out_hs = [
        nc.dram_tensor(f"out{b}", [128 * CS, 1], BF16, kind="ExternalOutput")
        for b in range(BT)
    ]
    with tile.TileContext(nc) as tc:
        _emit(nc, tc, embT_h, kern_h, labf_h, nrm_h, labrow_h,
              embrow_h, kncol_h, fix_h, out_hs)
    nc.compile()
    return nc


_NC = None


def _get_nc():
    global _NC
    if _NC is None:
        _NC = _build()
    return _NC


def _prep_inputs(embbedings, norms, label, kernel):
    import ml_dtypes

    bf16 = ml_dtypes.bfloat16
    emb = np.asarray(embbedings, dtype=np.float32)
    nrm = np.asarray(norms, dtype=np.float32).reshape(B)
    lab = np.asarray(label).astype(np.int64).reshape(B)
    kern = np.asarray(kernel, dtype=np.float32)

    # fold column normalization and the S scale into the bf16 kernel
    colnorm = np.sqrt((kern * kern).sum(axis=0))
    knS = np.zeros((EMB, CS * NCORES), dtype=np.float32)
    knS[:, :C] = kern * (S / colnorm)
    knS16 = knS.astype(bf16)

    e16 = emb.astype(bf16)
    embT_arr = np.ascontiguousarray(
        e16.T.reshape(KT, 128, B).transpose(1, 0, 2).reshape(128, KT * B)
    )
    embrow_arr = np.ascontiguousarray(
        e16.reshape(BT, 128, EMB).transpose(1, 0, 2).reshape(128, BT * EMB)
    )
    kncg = np.ascontiguousarray(knS16[:, lab].T)  # [B, EMB]
    kncol_arr = np.ascontiguousarray(
        kncg.reshape(BT, 128, EMB).transpose(1, 0, 2).reshape(128, BT * EMB)
    )
    nrm_arr = np.ascontiguousarray(nrm.reshape(BT, 128).T)

    blocks = _blocks()
    in_maps = []
    for c in range(NCORES):
        kc4 = knS16[:, c * CS : (c + 1) * CS].reshape(KT, 128, CS)
        kern_arr = np.concatenate(
            [
                kc4[:, :, c0 : c0 + Wb].transpose(1, 0, 2).reshape(128, KT * Wb)
                for (c0, Wb, _) in blocks
            ],
            axis=1,
        )
        la = (lab - c * CS).astype(np.int32)
        labf_arr = np.ascontiguousarray(la.reshape(BT, 128).T).astype(np.float32)
        labrow_arr = la.astype(np.float32).reshape(1, B)
        in_maps.append(
            {
                "embT": embT_arr,
                "kern": np.ascontiguousarray(kern_arr),
                "labf": labf_arr,
                "nrm": nrm_arr,
                "labrow": labrow_arr,
                "embrow": embrow_arr,
                "kncol": kncol_arr,
            }
        )
    return in_maps, lab


def _run(in_maps, **kwargs):
    nc = _get_nc()
    return run_bass_kernel_spmd(nc, in_maps, core_ids=list(range(NCORES)), **kwargs)


def _assemble(res, lab):
    parts = []
    for c in range(NCORES):
        rows = [res.results[c][f"out{b}"].reshape(128, CS) for b in range(BT)]
        parts.append(np.concatenate(rows, axis=0))
    out = np.concatenate(parts, axis=1)[:, :C].astype(np.float32)
    # place the device-computed margin fix-up values at (i, label_i)
    fix = np.asarray(res.results[0]["fix"], dtype=np.float32)  # [128, BT]
    out[np.arange(B), lab] = fix.T.reshape(B)
    return out


def kernel(embbedings, norms, label, kernel):
    in_maps, lab = _prep_inputs(embbedings, norms, label, kernel)
    res = _run(in_maps)
    return _assemble(res, lab)


# revision 19
# speedup vs baseline: 1.0717x; 1.0717x over previous
"""CWCFace head (nn_CWCFace_11201274708637) — Trainium2 Bass kernel.

Math (reference):
    kn = kernel / ||kernel||_col
    cos = clip(emb @ kn, -1+eps, 1-eps)              # [B, C]
    ms  = margin_scaler(norms, label)                # [B, 1] per-sample stats
    th  = arccos(cos); th_m = clip(th + onehot*(-M*ms), eps, pi-eps)
    out = (cos(th_m) - onehot*(M + M*ms)) * S

The onehot terms touch exactly ONE column per row, so the full [B, C]
tensor only needs  out = clip(S*cos)  plus a B-element fix-up at
(i, label_i).  cos(th+g) for those B elements uses the identity
cos(th+g) = t*cos(g) - sqrt(1-t^2)*sin(g); the theta-clip branches are
threshold comparisons — no arccos.

Device kernel = one big bf16 matmul + clamp + scatter fix-up:
  - the column normalization and the S scale are folded into the bf16
    kernel upload (host-side input prep), so psum is S*cos directly
  - epilogue is a single DVE tensor_scalar clamp PSUM(f32) -> SBUF bf16,
    output DMA is bf16 (half the write traffic of f32)
  - fix-up values come from per-sample dot products against the label's
    kernel column (host gathers the columns; device does the math), so
    no gather-after-store serial tail — just 4 indirect scatters that
    wait on the stores of their B-tile
  - margin stats (segment count/sum/sumsq) via BxB label-equality
    matmul, as before

Sharding: classes column-split over 8 cores, CS=8848 per core
(8*8848 = 70784 >= 70722).  Kernel blocks are uploaded pre-swizzled to
[128, KT*W] so every block load is one long contiguous DMA per
partition; blocks are prefetched 2 ahead to keep the PE gap-free (and
at the max p-state).
"""

import sys

for _p in (
    "/root/.axon_site",
    "/root/.axon_site/_ro/trn_rl_repo",
    "/root/.axon_site/_ro/pypackages",
    "/opt/trn_rl_repo",
):
    if _p not in sys.path:
        sys.path.append(_p)

import math

import numpy as np

import concourse.bass as bass
import concourse.mybir as mybir
import concourse.tile as tile
from concourse import bacc
from concourse.bass import IndirectOffsetOnAxis
from concourse.bass_utils import run_bass_kernel_spmd

B = 512
EMB = 512
C = 70722
NCORES = 8
CS = 8848  # per-core classes (padded);  8 * 8848 = 70784 >= 70722
S = 64.0
MARG = 0.4
H = 0.333
EPS = 1e-3

F32 = mybir.dt.float32
BF16 = mybir.dt.bfloat16
I32 = mybir.dt.int32
AL = mybir.AluOpType
AF = mybir.ActivationFunctionType

KT = EMB // 128          # 4 K-tiles
BT = B // 128            # 4 B-tiles
COS_EPS = float(math.cos(EPS))
PI_2 = math.pi / 2.0
CLAMP = S * (1.0 - EPS)


def _rounds():
    """Compute rounds: (c0, W).  W=1024 fills half of PSUM per B-tile so
    two rounds can be in flight; the short tail round drains fast."""
    out = []
    c0 = 0
    while c0 < CS:
        w = min(1024, CS - c0)
        out.append((c0, w))
        c0 += w
    return out


def _emit(nc, tc, embT_h, kern_h, labf_h, nrm_h, labrow_h,
          embrow_h, kncol_h, fix_h, out_hs):
    out2ds = [
        oh[:, :].rearrange("(p c) o -> p (c o)", c=CS) for oh in out_hs
    ]  # [128, CS] each

    cst_cm = tc.tile_pool(name="cst", bufs=1)
    cst = cst_cm.__enter__()

    embT_sb = cst.tile([128, KT, B], BF16, tag="embT")      # [p, k, b]
    embrow_sb = cst.tile([128, BT, EMB], BF16, tag="embrow")
    kncol_sb = cst.tile([128, BT, EMB], BF16, tag="kncol")
    labf_sb = cst.tile([128, BT], F32, tag="labf")
    nrm_sb = cst.tile([128, BT], F32, tag="nrm")
    labrow_sb = cst.tile([1, B], F32, tag="labrow")
    g_sb = cst.tile([128, BT], F32, tag="g")        # -M * ms
    gadd_sb = cst.tile([128, BT], F32, tag="gadd")  # M + M * ms
    v_sb = cst.tile([128, BT], F32, tag="v")        # safe norms

    def load_embT():
        # split by k so the very first matmul can start after ~160KB
        for k in range(KT):
            nc.sync.dma_start(
                out=embT_sb[:, k, :], in_=embT_h[:, k * B : (k + 1) * B]
            )

    def load_small():
        nc.sync.dma_start(out=embrow_sb[:], in_=embrow_h[:, :].rearrange(
            "p (t e) -> p t e", t=BT))
        nc.sync.dma_start(out=kncol_sb[:], in_=kncol_h[:, :].rearrange(
            "p (t e) -> p t e", t=BT))
        nc.sync.dma_start(out=labf_sb[:], in_=labf_h[:, :])
        nc.sync.dma_start(out=nrm_sb[:], in_=nrm_h[:, :])
        nc.sync.dma_start(out=labrow_sb[:], in_=labrow_h[:, :])

    rounds = _rounds()
    kernR = kern_h  # [128, KT*CS], round r at columns KT*c0 .. KT*(c0+W)

    with (
        tc.tile_pool(name="pa", bufs=2) as pa,
        tc.tile_pool(name="kp", bufs=4) as kp,
        tc.tile_pool(name="op", bufs=2) as op_,
        tc.tile_pool(name="ps", bufs=4, space="PSUM") as ps,
        tc.tile_pool(name="pc", bufs=1) as pc,
    ):
        def load_round(ri, split_k=False):
            c0, W = rounds[ri]
            ksb = kp.tile([128, KT, W], BF16, tag="ks")
            src = kernR[:, KT * c0 : KT * (c0 + W)].rearrange(
                "p (k w) -> p k w", k=KT
            )
            if split_k:
                for k in range(KT):
                    nc.sync.dma_start(out=ksb[:, k, :], in_=src[:, k, :])
            else:
                nc.sync.dma_start(out=ksb[:], in_=src)
            return ksb

        def main_round(ri, ksb, osbs):
            """One round: 4 B-tiles x [128, W] psum, one clamp per B-tile.
            Clamped bf16 goes into the round-pair staging tile osbs[b]; the
            [128, 2048] store is issued (from the otherwise-idle Scalar
            queue) when the pair completes."""
            c0, W = rounds[ri]
            half = (ri % 2) * 1024
            for b in range(BT):
                ps_out = ps.tile([128, W], F32, space="PSUM", tag="po")
                for k in range(KT):
                    for j in range(0, W, 512):
                        wj = min(512, W - j)
                        nc.tensor.matmul(
                            ps_out[:, j : j + wj],
                            embT_sb[:, k, b * 128 : (b + 1) * 128],
                            ksb[:, k, j : j + wj],
                            start=(k == 0),
                            stop=(k == KT - 1),
                        )
                nc.vector.tensor_scalar(
                    osbs[b][:, half : half + W], ps_out[:],
                    -CLAMP, CLAMP, op0=AL.max, op1=AL.min,
                )
                if ri == len(rounds) - 1 or ri % 2 == 1:
                    p0 = c0 - half
                    eng = nc.scalar if b % 2 == 0 else nc.sync
                    eng.dma_start(
                        out=out2ds[b][:, p0 : p0 + half + W],
                        in_=osbs[b][:, : half + W],
                    )

        def phase_a():
            """Margin-scaler segment stats -> g_sb, gadd_sb."""
            labAll = pa.tile([128, B], F32, tag="labAll")
            nc.gpsimd.partition_broadcast(labAll[:], labrow_sb[:])

            nc.vector.tensor_scalar(
                v_sb[:], nrm_sb[:], 0.001, 100.0, op0=AL.max, op1=AL.min
            )
            w_sb = pa.tile([128, 3 * BT], F32, tag="w")
            nc.vector.memset(w_sb[:], 1.0)
            for b in range(BT):
                nc.vector.tensor_copy(
                    w_sb[:, 3 * b + 1 : 3 * b + 2], v_sb[:, b : b + 1]
                )
                nc.vector.tensor_tensor(
                    out=w_sb[:, 3 * b + 2 : 3 * b + 3],
                    in0=v_sb[:, b : b + 1],
                    in1=v_sb[:, b : b + 1],
                    op=AL.mult,
                )

            st_all = pa.tile([128, 3 * BT], F32, tag="st_all")
            for a in range(BT):
                ps_st = ps.tile([128, 3], F32, space="PSUM", tag="po")
                for b in range(BT):
                    eq = pa.tile([128, 128], F32, tag="eq")
                    nc.vector.tensor_tensor(
                        out=eq[:],
                        in0=labf_sb[:, b : b + 1].to_broadcast([128, 128]),
                        in1=labAll[:, a * 128 : (a + 1) * 128],
                        op=AL.is_equal,
                    )
                    nc.tensor.matmul(
                        ps_st[:],
                        eq[:],
                        w_sb[:, 3 * b : 3 * b + 3],
                        start=(b == 0),
                        stop=(b == BT - 1),
                    )
                nc.vector.tensor_copy(st_all[:, 3 * a : 3 * a + 3], ps_st[:])

            stv = st_all[:].rearrange("p (a c) -> p a c", c=3)
            n_ = stv[:, :, 0]
            sm = stv[:, :, 1]
            sq2 = stv[:, :, 2]

            t0 = pa.tile([128, 8 * BT], F32, tag="t0")
            tv = t0[:].rearrange("p (i a) -> p i a", a=BT)
            rn = tv[:, 0, :]
            nc.vector.reciprocal(rn, n_)
            mean = tv[:, 1, :]
            nc.vector.tensor_tensor(out=mean, in0=sm, in1=rn, op=AL.mult)
            m2 = tv[:, 2, :]
            nc.vector.tensor_tensor(out=m2, in0=mean, in1=mean, op=AL.mult)
            nm2 = tv[:, 3, :]
            nc.vector.tensor_tensor(out=nm2, in0=n_, in1=m2, op=AL.mult)
            num = tv[:, 4, :]
            nc.vector.tensor_tensor(out=num, in0=sq2, in1=nm2, op=AL.subtract)
            den = tv[:, 5, :]
            nc.vector.tensor_scalar(den, n_, -1.0, 1.0, op0=AL.add, op1=AL.max)
            rden = tv[:, 6, :]
            nc.vector.reciprocal(rden, den)
            var = tv[:, 7, :]
            nc.vector.tensor_tensor(out=var, in0=num, in1=rden, op=AL.mult)
            nc.vector.tensor_scalar(var, var, 1e-30, None, op0=AL.max)

            t1 = pa.tile([128, 8 * BT], F32, tag="t1")
            uv = t1[:].rearrange("p (i a) -> p i a", a=BT)
            ars = uv[:, 0, :]
            nc.scalar.activation(ars, var, AF.Abs_reciprocal_sqrt)
            std = uv[:, 1, :]
            nc.vector.tensor_tensor(out=std, in0=var, in1=ars, op=AL.mult)
            stdp = uv[:, 2, :]
            nc.vector.tensor_scalar(stdp, std, EPS, None, op0=AL.add)
            rstd = uv[:, 3, :]
            nc.vector.reciprocal(rstd, stdp)
            mask = uv[:, 4, :]
            nc.vector.tensor_scalar(mask, n_, 2.0, None, op0=AL.is_gt)
            mask_i = pa.tile([128, BT], I32, tag="mask_i")
            nc.vector.tensor_copy(mask_i[:], mask)
            c05 = uv[:, 5, :]
            nc.vector.memset(c05, 0.05)
            invd = uv[:, 6, :]
            nc.vector.select(invd, mask_i[:], rstd, c05)
            dv = uv[:, 7, :]
            nc.vector.tensor_tensor(out=dv, in0=v_sb[:], in1=mean, op=AL.subtract)
            res = tv[:, 0, :]
            nc.vector.tensor_tensor(out=res, in0=dv, in1=invd, op=AL.mult)
            ms = tv[:, 1, :]
            nc.vector.tensor_scalar(ms, res, H, 1.0, op0=AL.mult, op1=AL.min)
            nc.vector.tensor_scalar(ms, ms, -1.0, None, op0=AL.max)
            nc.vector.tensor_scalar(g_sb[:], ms, -MARG, None, op0=AL.mult)
            nc.vector.tensor_scalar(
                gadd_sb[:], ms, MARG, MARG, op0=AL.mult, op1=AL.add
            )

        def phase_c_pre():
            """Angle thresholds (needs g_sb)."""
            pcst = {}
            cpi2 = pc.tile([128, 1], F32, tag="cpi2")
            nc.vector.memset(cpi2[:], PI_2)
            cpie = pc.tile([128, 1], F32, tag="cpie")
            nc.vector.memset(cpie[:], PI_2 + EPS)

            cosg = pc.tile([128, BT], F32, tag="cosg")
            sing = pc.tile([128, BT], F32, tag="sing")
            thr_lo = pc.tile([128, BT], F32, tag="thr_lo")
            thr_hi = pc.tile([128, BT], F32, tag="thr_hi")
            for b in range(BT):
                gb = g_sb[:, b : b + 1]
                nc.scalar.activation(cosg[:, b : b + 1], gb, AF.Sin, bias=cpi2[:])
                nc.scalar.activation(sing[:, b : b + 1], gb, AF.Sin)
                nc.scalar.activation(
                    thr_lo[:, b : b + 1], gb, AF.Sin, bias=cpie[:], scale=-1.0
                )
                nc.scalar.activation(
                    thr_hi[:, b : b + 1], gb, AF.Sin, bias=cpie[:], scale=1.0
                )
            nthr = pc.tile([128, BT], F32, tag="nthr")
            nc.vector.tensor_scalar(nthr[:], thr_hi[:], -1.0, None, op0=AL.mult)
            ml1 = pc.tile([128, BT], F32, tag="ml1")
            nc.vector.tensor_scalar(ml1[:], g_sb[:], EPS, None, op0=AL.is_lt)
            mh1 = pc.tile([128, BT], F32, tag="mh1")
            nc.vector.tensor_scalar(mh1[:], g_sb[:], -EPS, None, op0=AL.is_gt)
            c_lo = pc.tile([128, BT], F32, tag="c_lo")
            nc.vector.memset(c_lo[:], COS_EPS)
            c_hi = pc.tile([128, BT], F32, tag="c_hi")
            nc.vector.memset(c_hi[:], -COS_EPS)
            pcst.update(
                cosg=cosg, sing=sing, thr_lo=thr_lo, nthr=nthr,
                ml1=ml1, mh1=mh1, c_lo=c_lo, c_hi=c_hi,
            )
            return pcst

        def phase_c_vals(pcst):
            """Fix-up values from direct dot products (no HBM gather)."""
            traw = pc.tile([128, BT], F32, tag="traw")
            scr = pc.tile([128, EMB], F32, tag="scr")
            for b in range(BT):
                nc.vector.scalar_tensor_tensor(
                    out=scr[:],
                    in0=embrow_sb[:, b, :],
                    scalar=1.0,
                    in1=kncol_sb[:, b, :],
                    op0=AL.mult,
                    op1=AL.mult,
                    accum_out=traw[:, b : b + 1],
                )
            t_ = pc.tile([128, BT], F32, tag="t_")
            nc.vector.tensor_scalar(
                t_[:], traw[:], 1.0 / S, 1.0 - EPS, op0=AL.mult, op1=AL.min
            )
            nc.vector.tensor_scalar(t_[:], t_[:], -1.0 + EPS, None, op0=AL.max)

            t2 = pc.tile([128, BT], F32, tag="t2")
            nc.vector.tensor_tensor(out=t2[:], in0=t_[:], in1=t_[:], op=AL.mult)
            om = pc.tile([128, BT], F32, tag="om")
            nc.vector.tensor_scalar(om[:], t2[:], -1.0, 1.0, op0=AL.mult, op1=AL.add)
            omr = pc.tile([128, BT], F32, tag="omr")
            nc.scalar.activation(omr[:], om[:], AF.Abs_reciprocal_sqrt)
            sq = pc.tile([128, BT], F32, tag="sq")
            nc.vector.tensor_tensor(out=sq[:], in0=om[:], in1=omr[:], op=AL.mult)

            a1 = pc.tile([128, BT], F32, tag="a1")
            nc.vector.tensor_tensor(out=a1[:], in0=t_[:], in1=pcst["cosg"][:], op=AL.mult)
            a2 = pc.tile([128, BT], F32, tag="a2")
            nc.vector.tensor_tensor(out=a2[:], in0=sq[:], in1=pcst["sing"][:], op=AL.mult)
            cosm = pc.tile([128, BT], F32, tag="cosm")
            nc.vector.tensor_tensor(out=cosm[:], in0=a1[:], in1=a2[:], op=AL.subtract)

            ml2 = pc.tile([128, BT], F32, tag="ml2")
            nc.vector.tensor_tensor(
                out=ml2[:], in0=t_[:], in1=pcst["thr_lo"][:], op=AL.is_gt
            )
            mlow = pc.tile([128, BT], F32, tag="mlow")
            nc.vector.tensor_tensor(out=mlow[:], in0=pcst["ml1"][:], in1=ml2[:], op=AL.mult)
            mh2 = pc.tile([128, BT], F32, tag="mh2")
            nc.vector.tensor_tensor(
                out=mh2[:], in0=t_[:], in1=pcst["nthr"][:], op=AL.is_lt
            )
            mhigh = pc.tile([128, BT], F32, tag="mhigh")
            nc.vector.tensor_tensor(out=mhigh[:], in0=pcst["mh1"][:], in1=mh2[:], op=AL.mult)

            mlow_i = pc.tile([128, BT], I32, tag="mlow_i")
            nc.vector.tensor_copy(mlow_i[:], mlow[:])
            mhigh_i = pc.tile([128, BT], I32, tag="mhigh_i")
            nc.vector.tensor_copy(mhigh_i[:], mhigh[:])
            nc.vector.select(cosm[:], mlow_i[:], pcst["c_lo"][:], cosm[:])
            nc.vector.select(cosm[:], mhigh_i[:], pcst["c_hi"][:], cosm[:])

            val = pc.tile([128, BT], F32, tag="val")
            nc.vector.tensor_tensor(
                out=val[:], in0=cosm[:], in1=gadd_sb[:], op=AL.subtract
            )
            nc.vector.tensor_scalar(val[:], val[:], S, None, op0=AL.mult)
            nc.scalar.dma_start(out=fix_h[:, :], in_=val[:])

        # ---- emission ----
        load_small()
        load_embT()
        ksbs = {0: load_round(0, split_k=True), 1: load_round(1),
                2: load_round(2)}
        phase_a()
        pcst = phase_c_pre()

        loaded = 3
        osbs = None
        for ri in range(len(rounds)):
            while loaded < len(rounds) and loaded <= ri + 3:
                ksbs[loaded] = load_round(loaded)
                loaded += 1
            if ri % 2 == 0:
                osbs = [
                    op_.tile([128, 2048], BF16, tag=f"o{b}", name=f"o{b}")
                    for b in range(BT)
                ]
            main_round(ri, ksbs[ri], osbs)
            if ri == 0:
                phase_c_vals(pcst)

    cst_cm.__exit__(None, None, None)


def _build():
    nc = bacc.Bacc(
        "TRN2", target_bir_lowering=False, debug=False, num_devices=NCORES
    )
    embT_h = nc.dram_tensor("embT", [128, KT * B], BF16, kind="ExternalInput")
    kern_h = nc.dram_tensor("kern", [128, KT * CS], BF16, kind="ExternalInput")
    labf_h = nc.dram_tensor("labf", [128, BT], F32, kind="ExternalInput")
    nrm_h = nc.dram_tensor("nrm", [128, BT], F32, kind="ExternalInput")
    labrow_h = nc.dram_tensor("labrow", [1, B], F32, kind="ExternalInput")
    embrow_h = nc.dram_tensor("embrow", [128, BT * EMB], BF16, kind="ExternalInput")
    kncol_h = nc.dram_tensor("kncol", [128, BT * EMB], BF16, kind="ExternalInput")
    fix_h = nc.dram_tensor("fix", [128, BT], F32, kind="ExternalOutput")
    out_hs = [
        nc.dram_tensor(f"out{b}", [128 * CS, 1], BF16, kind="ExternalOutput")
        for b in range(BT)
    ]
    with tile.TileContext(nc) as tc:
        _emit(nc, tc, embT_h, kern_h, labf_h, nrm_h, labrow_h,
              embrow_h, kncol_h, fix_h, out_hs)
    nc.compile()
    return nc


_NC = None


def _get_nc():
    global _NC
    if _NC is None:
        _NC = _build()
    return _NC


def _prep_inputs(embbedings, norms, label, kernel):
    import ml_dtypes

    bf16 = ml_dtypes.bfloat16
    emb = np.asarray(embbedings, dtype=np.float32)
    nrm = np.asarray(norms, dtype=np.float32).reshape(B)
    lab = np.asarray(label).astype(np.int64).reshape(B)
    kern = np.asarray(kernel, dtype=np.float32)

    # fold column normalization and the S scale into the bf16 kernel
    colnorm = np.sqrt((kern * kern).sum(axis=0))
    knS = np.zeros((EMB, CS * NCORES), dtype=np.float32)
    knS[:, :C] = kern * (S / colnorm)
    knS16 = knS.astype(bf16)

    e16 = emb.astype(bf16)
    embT_arr = np.ascontiguousarray(
        e16.T.reshape(KT, 128, B).transpose(1, 0, 2).reshape(128, KT * B)
    )
    embrow_arr = np.ascontiguousarray(
        e16.reshape(BT, 128, EMB).transpose(1, 0, 2).reshape(128, BT * EMB)
    )
    kncg = np.ascontiguousarray(knS16[:, lab].T)  # [B, EMB]
    kncol_arr = np.ascontiguousarray(
        kncg.reshape(BT, 128, EMB).transpose(1, 0, 2).reshape(128, BT * EMB)
    )
    nrm_arr = np.ascontiguousarray(nrm.reshape(BT, 128).T)

    rounds = _rounds()
    in_maps = []
    for c in range(NCORES):
        kc4 = knS16[:, c * CS : (c + 1) * CS].reshape(KT, 128, CS)
        kern_arr = np.concatenate(
            [
                kc4[:, :, c0 : c0 + W].transpose(1, 0, 2).reshape(128, KT * W)
                for (c0, W) in rounds
            ],
            axis=1,
        )
        la = (lab - c * CS).astype(np.int32)
        labf_arr = np.ascontiguousarray(la.reshape(BT, 128).T).astype(np.float32)
        labrow_arr = la.astype(np.float32).reshape(1, B)
        in_maps.append(
            {
                "embT": embT_arr,
                "kern": np.ascontiguousarray(kern_arr),
                "labf": labf_arr,
                "nrm": nrm_arr,
                "labrow": labrow_arr,
                "embrow": embrow_arr,
                "kncol": kncol_arr,
            }
        )
    return in_maps, lab


def _run(in_maps, **kwargs):
    nc = _get_nc()
    return run_bass_kernel_spmd(nc, in_maps, core_ids=list(range(NCORES)), **kwargs)


def _assemble(res, lab):
    parts = []
    for c in range(NCORES):
        rows = [res.results[c][f"out{b}"].reshape(128, CS) for b in range(BT)]
        parts.append(np.concatenate(rows, axis=0))
    out = np.concatenate(parts, axis=1)[:, :C].astype(np.float32)
    # place the device-computed margin fix-up values at (i, label_i)
    fix = np.asarray(res.results[0]["fix"], dtype=np.float32)  # [128, BT]
    out[np.arange(B), lab] = fix.T.reshape(B)
    return out


def kernel(embbedings, norms, label, kernel):
    in_maps, lab = _prep_inputs(embbedings, norms, label, kernel)
    res = _run(in_maps)
    return _assemble(res, lab)


# revision 27
# speedup vs baseline: 1.0971x; 1.0237x over previous
"""CWCFace head (nn_CWCFace_11201274708637) — Trainium2 Bass kernel.

Math (reference):
    kn = kernel / ||kernel||_col
    cos = clip(emb @ kn, -1+eps, 1-eps)              # [B, C]
    ms  = margin_scaler(norms, label)                # [B, 1] per-sample stats
    th  = arccos(cos); th_m = clip(th + onehot*(-M*ms), eps, pi-eps)
    out = (cos(th_m) - onehot*(M + M*ms)) * S

The onehot terms touch exactly ONE column per row, so the full [B, C]
tensor only needs  out = clip(S*cos)  plus a B-element fix-up at
(i, label_i).  cos(th+g) for those B elements uses the identity
cos(th+g) = t*cos(g) - sqrt(1-t^2)*sin(g); the theta-clip branches are
threshold comparisons — no arccos.

Device kernel = one big bf16 matmul + clamp + scatter fix-up:
  - the column normalization and the S scale are folded into the bf16
    kernel upload (host-side input prep), so psum is S*cos directly
  - epilogue is a single DVE tensor_scalar clamp PSUM(f32) -> SBUF bf16,
    output DMA is bf16 (half the write traffic of f32)
  - fix-up values come from per-sample dot products against the label's
    kernel column (host gathers the columns; device does the math), so
    no gather-after-store serial tail — just 4 indirect scatters that
    wait on the stores of their B-tile
  - margin stats (segment count/sum/sumsq) via BxB label-equality
    matmul, as before

Sharding: classes column-split over 8 cores, CS=8848 per core
(8*8848 = 70784 >= 70722).  Kernel blocks are uploaded pre-swizzled to
[128, KT*W] so every block load is one long contiguous DMA per
partition; blocks are prefetched 2 ahead to keep the PE gap-free (and
at the max p-state).
"""

import sys

for _p in (
    "/root/.axon_site",
    "/root/.axon_site/_ro/trn_rl_repo",
    "/root/.axon_site/_ro/pypackages",
    "/opt/trn_rl_repo",
):
    if _p not in sys.path:
        sys.path.append(_p)

import math

import numpy as np

import concourse.bass as bass
import concourse.mybir as mybir
import concourse.tile as tile
from concourse import bacc
from concourse.bass import IndirectOffsetOnAxis
from concourse.bass_utils import run_bass_kernel_spmd

B = 512
EMB = 512
C = 70722
NCORES = 8
CS = 8848  # per-core classes (padded);  8 * 8848 = 70784 >= 70722
S = 64.0
MARG = 0.4
H = 0.333
EPS = 1e-3

F32 = mybir.dt.float32
BF16 = mybir.dt.bfloat16
I32 = mybir.dt.int32
AL = mybir.AluOpType
AF = mybir.ActivationFunctionType

KT = EMB // 128          # 4 K-tiles
BT = B // 128            # 4 B-tiles
COS_EPS = float(math.cos(EPS))
PI_2 = math.pi / 2.0
CLAMP = S * (1.0 - EPS)


def _rounds():
    """Compute rounds: (c0, W).  W=1024 fills half of PSUM per B-tile so
    two rounds can be in flight; the short tail round drains fast."""
    out = []
    c0 = 0
    while c0 < CS:
        w = min(1024, CS - c0)
        out.append((c0, w))
        c0 += w
    return out


def _emit(nc, tc, embT_h, kern_h, labf_h, nrm_h, labrow_h,
          embrow_h, kncol_h, fix_h, out_hs):
    out2ds = [
        oh[:, :].rearrange("(p c) o -> p (c o)", c=CS) for oh in out_hs
    ]  # [128, CS] each

    cst_cm = tc.tile_pool(name="cst", bufs=1)
    cst = cst_cm.__enter__()

    embT_sb = cst.tile([128, KT, B], BF16, tag="embT")      # [p, k, b]
    embrow_sb = cst.tile([128, BT, EMB], BF16, tag="embrow")
    kncol_sb = cst.tile([128, BT, EMB], BF16, tag="kncol")
    labf_sb = cst.tile([128, BT], F32, tag="labf")
    nrm_sb = cst.tile([128, BT], F32, tag="nrm")
    labrow_sb = cst.tile([1, B], F32, tag="labrow")
    g_sb = cst.tile([128, BT], F32, tag="g")        # -M * ms
    gadd_sb = cst.tile([128, BT], F32, tag="gadd")  # M + M * ms
    v_sb = cst.tile([128, BT], F32, tag="v")        # safe norms

    def load_embT():
        # split by k so the very first matmul can start after ~160KB
        for k in range(KT):
            nc.sync.dma_start(
                out=embT_sb[:, k, :], in_=embT_h[:, k * B : (k + 1) * B]
            )

    def load_small():
        # tiny tensors first: phase A can start the moment these land
        nc.sync.dma_start(out=labrow_sb[:], in_=labrow_h[:, :])
        nc.sync.dma_start(out=labf_sb[:], in_=labf_h[:, :])
        nc.sync.dma_start(out=nrm_sb[:], in_=nrm_h[:, :])

    def load_rowcols():
        # only needed by phase_c_vals (~25us in) — keep off the start path
        nc.sync.dma_start(out=embrow_sb[:], in_=embrow_h[:, :].rearrange(
            "p (t e) -> p t e", t=BT))
        nc.sync.dma_start(out=kncol_sb[:], in_=kncol_h[:, :].rearrange(
            "p (t e) -> p t e", t=BT))

    rounds = _rounds()
    kernR = kern_h  # [128, KT*CS], round r at columns KT*c0 .. KT*(c0+W)

    with (
        tc.tile_pool(name="pa", bufs=2) as pa,
        tc.tile_pool(name="kp", bufs=4) as kp,
        tc.tile_pool(name="op", bufs=2) as op_,
        tc.tile_pool(name="ps", bufs=4, space="PSUM") as ps,
        tc.tile_pool(name="pc", bufs=1) as pc,
    ):
        def load_round(ri, split_k=False):
            c0, W = rounds[ri]
            ksb = kp.tile([128, KT, W], BF16, tag="ks")
            src = kernR[:, KT * c0 : KT * (c0 + W)].rearrange(
                "p (k w) -> p k w", k=KT
            )
            if split_k:
                for k in range(KT):
                    for j in range(0, W, 512):
                        wj = min(512, W - j)
                        nc.sync.dma_start(
                            out=ksb[:, k, j : j + wj], in_=src[:, k, j : j + wj]
                        )
            else:
                nc.sync.dma_start(out=ksb[:], in_=src)
            return ksb

        def main_round(ri, ksb, osbs):
            """One round: 4 B-tiles x [128, W] psum, one clamp per B-tile.
            Clamped bf16 goes into the round-pair staging tile osbs[b]; the
            [128, 2048] store is issued (from the otherwise-idle Scalar
            queue) when the pair completes."""
            c0, W = rounds[ri]
            half = (ri % 2) * 1024
            for b in range(BT):
                ps_out = ps.tile([128, W], F32, space="PSUM", tag="po")
                for k in range(KT):
                    for j in range(0, W, 512):
                        wj = min(512, W - j)
                        nc.tensor.matmul(
                            ps_out[:, j : j + wj],
                            embT_sb[:, k, b * 128 : (b + 1) * 128],
                            ksb[:, k, j : j + wj],
                            start=(k == 0),
                            stop=(k == KT - 1),
                        )
                nc.vector.tensor_scalar(
                    osbs[b][:, half : half + W], ps_out[:],
                    -CLAMP, CLAMP, op0=AL.max, op1=AL.min,
                )
                if ri == len(rounds) - 1 or ri % 2 == 1:
                    p0 = c0 - half
                    eng = nc.scalar if b % 2 == 0 else nc.sync
                    eng.dma_start(
                        out=out2ds[b][:, p0 : p0 + half + W],
                        in_=osbs[b][:, : half + W],
                    )

        def phase_a():
            """Margin-scaler segment stats -> g_sb, gadd_sb."""
            labAll = pa.tile([128, B], F32, tag="labAll")
            nc.gpsimd.partition_broadcast(labAll[:], labrow_sb[:])

            nc.vector.tensor_scalar(
                v_sb[:], nrm_sb[:], 0.001, 100.0, op0=AL.max, op1=AL.min
            )
            w_sb = pa.tile([128, 3 * BT], F32, tag="w")
            nc.vector.memset(w_sb[:], 1.0)
            for b in range(BT):
                nc.vector.tensor_copy(
                    w_sb[:, 3 * b + 1 : 3 * b + 2], v_sb[:, b : b + 1]
                )
                nc.vector.tensor_tensor(
                    out=w_sb[:, 3 * b + 2 : 3 * b + 3],
                    in0=v_sb[:, b : b + 1],
                    in1=v_sb[:, b : b + 1],
                    op=AL.mult,
                )

            st_all = pa.tile([128, 3 * BT], F32, tag="st_all")
            for a in range(BT):
                ps_st = ps.tile([128, 3], F32, space="PSUM", tag="po")
                for b in range(BT):
                    eq = pa.tile([128, 128], F32, tag="eq")
                    nc.vector.tensor_tensor(
                        out=eq[:],
                        in0=labf_sb[:, b : b + 1].to_broadcast([128, 128]),
                        in1=labAll[:, a * 128 : (a + 1) * 128],
                        op=AL.is_equal,
                    )
                    nc.tensor.matmul(
                        ps_st[:],
                        eq[:],
                        w_sb[:, 3 * b : 3 * b + 3],
                        start=(b == 0),
                        stop=(b == BT - 1),
                    )
                nc.vector.tensor_copy(st_all[:, 3 * a : 3 * a + 3], ps_st[:])

            stv = st_all[:].rearrange("p (a c) -> p a c", c=3)
            n_ = stv[:, :, 0]
            sm = stv[:, :, 1]
            sq2 = stv[:, :, 2]

            t0 = pa.tile([128, 8 * BT], F32, tag="t0")
            tv = t0[:].rearrange("p (i a) -> p i a", a=BT)
            rn = tv[:, 0, :]
            nc.vector.reciprocal(rn, n_)
            mean = tv[:, 1, :]
            nc.vector.tensor_tensor(out=mean, in0=sm, in1=rn, op=AL.mult)
            m2 = tv[:, 2, :]
            nc.vector.tensor_tensor(out=m2, in0=mean, in1=mean, op=AL.mult)
            nm2 = tv[:, 3, :]
            nc.vector.tensor_tensor(out=nm2, in0=n_, in1=m2, op=AL.mult)
            num = tv[:, 4, :]
            nc.vector.tensor_tensor(out=num, in0=sq2, in1=nm2, op=AL.subtract)
            den = tv[:, 5, :]
            nc.vector.tensor_scalar(den, n_, -1.0, 1.0, op0=AL.add, op1=AL.max)
            rden = tv[:, 6, :]
            nc.vector.reciprocal(rden, den)
            var = tv[:, 7, :]
            nc.vector.tensor_tensor(out=var, in0=num, in1=rden, op=AL.mult)
            nc.vector.tensor_scalar(var, var, 1e-30, None, op0=AL.max)

            t1 = pa.tile([128, 8 * BT], F32, tag="t1")
            uv = t1[:].rearrange("p (i a) -> p i a", a=BT)
            ars = uv[:, 0, :]
            nc.scalar.activation(ars, var, AF.Abs_reciprocal_sqrt)
            std = uv[:, 1, :]
            nc.vector.tensor_tensor(out=std, in0=var, in1=ars, op=AL.mult)
            stdp = uv[:, 2, :]
            nc.vector.tensor_scalar(stdp, std, EPS, None, op0=AL.add)
            rstd = uv[:, 3, :]
            nc.vector.reciprocal(rstd, stdp)
            mask = uv[:, 4, :]
            nc.vector.tensor_scalar(mask, n_, 2.0, None, op0=AL.is_gt)
            mask_i = pa.tile([128, BT], I32, tag="mask_i")
            nc.vector.tensor_copy(mask_i[:], mask)
            c05 = uv[:, 5, :]
            nc.vector.memset(c05, 0.05)
            invd = uv[:, 6, :]
            nc.vector.select(invd, mask_i[:], rstd, c05)
            dv = uv[:, 7, :]
            nc.vector.tensor_tensor(out=dv, in0=v_sb[:], in1=mean, op=AL.subtract)
            res = tv[:, 0, :]
            nc.vector.tensor_tensor(out=res, in0=dv, in1=invd, op=AL.mult)
            ms = tv[:, 1, :]
            nc.vector.tensor_scalar(ms, res, H, 1.0, op0=AL.mult, op1=AL.min)
            nc.vector.tensor_scalar(ms, ms, -1.0, None, op0=AL.max)
            nc.vector.tensor_scalar(g_sb[:], ms, -MARG, None, op0=AL.mult)
            nc.vector.tensor_scalar(
                gadd_sb[:], ms, MARG, MARG, op0=AL.mult, op1=AL.add
            )

        def phase_c_pre():
            """Angle thresholds (needs g_sb)."""
            pcst = {}
            cpi2 = pc.tile([128, 1], F32, tag="cpi2")
            nc.vector.memset(cpi2[:], PI_2)
            cpie = pc.tile([128, 1], F32, tag="cpie")
            nc.vector.memset(cpie[:], PI_2 + EPS)

            cosg = pc.tile([128, BT], F32, tag="cosg")
            sing = pc.tile([128, BT], F32, tag="sing")
            thr_lo = pc.tile([128, BT], F32, tag="thr_lo")
            thr_hi = pc.tile([128, BT], F32, tag="thr_hi")
            for b in range(BT):
                gb = g_sb[:, b : b + 1]
                nc.scalar.activation(cosg[:, b : b + 1], gb, AF.Sin, bias=cpi2[:])
                nc.scalar.activation(sing[:, b : b + 1], gb, AF.Sin)
                nc.scalar.activation(
                    thr_lo[:, b : b + 1], gb, AF.Sin, bias=cpie[:], scale=-1.0
                )
                nc.scalar.activation(
                    thr_hi[:, b : b + 1], gb, AF.Sin, bias=cpie[:], scale=1.0
                )
            nthr = pc.tile([128, BT], F32, tag="nthr")
            nc.vector.tensor_scalar(nthr[:], thr_hi[:], -1.0, None, op0=AL.mult)
            ml1 = pc.tile([128, BT], F32, tag="ml1")
            nc.vector.tensor_scalar(ml1[:], g_sb[:], EPS, None, op0=AL.is_lt)
            mh1 = pc.tile([128, BT], F32, tag="mh1")
            nc.vector.tensor_scalar(mh1[:], g_sb[:], -EPS, None, op0=AL.is_gt)
            c_lo = pc.tile([128, BT], F32, tag="c_lo")
            nc.vector.memset(c_lo[:], COS_EPS)
            c_hi = pc.tile([128, BT], F32, tag="c_hi")
            nc.vector.memset(c_hi[:], -COS_EPS)
            pcst.update(
                cosg=cosg, sing=sing, thr_lo=thr_lo, nthr=nthr,
                ml1=ml1, mh1=mh1, c_lo=c_lo, c_hi=c_hi,
            )
            return pcst

        def phase_c_dot(pcst, b):
            """One per-B-tile dot product vs the label's kernel column;
            spread across rounds so the DVE never falls behind the PE."""
            if "traw" not in pcst:
                pcst["traw"] = pc.tile([128, BT], F32, tag="traw", name="traw")
                pcst["scr"] = pc.tile([128, BT, EMB], F32, tag="scr", name="scr")
            nc.vector.scalar_tensor_tensor(
                out=pcst["scr"][:, b, :],
                in0=embrow_sb[:, b, :],
                scalar=1.0,
                in1=kncol_sb[:, b, :],
                op0=AL.mult,
                op1=AL.mult,
                accum_out=pcst["traw"][:, b : b + 1],
            )

        def phase_c_vals(pcst):
            """Fix-up values from direct dot products (no HBM gather)."""
            traw = pcst["traw"]
            t_ = pc.tile([128, BT], F32, tag="t_")
            nc.vector.tensor_scalar(
                t_[:], traw[:], 1.0 / S, 1.0 - EPS, op0=AL.mult, op1=AL.min
            )
            nc.vector.tensor_scalar(t_[:], t_[:], -1.0 + EPS, None, op0=AL.max)

            t2 = pc.tile([128, BT], F32, tag="t2")
            nc.vector.tensor_tensor(out=t2[:], in0=t_[:], in1=t_[:], op=AL.mult)
            om = pc.tile([128, BT], F32, tag="om")
            nc.vector.tensor_scalar(om[:], t2[:], -1.0, 1.0, op0=AL.mult, op1=AL.add)
            omr = pc.tile([128, BT], F32, tag="omr")
            nc.scalar.activation(omr[:], om[:], AF.Abs_reciprocal_sqrt)
            sq = pc.tile([128, BT], F32, tag="sq")
            nc.vector.tensor_tensor(out=sq[:], in0=om[:], in1=omr[:], op=AL.mult)

            a1 = pc.tile([128, BT], F32, tag="a1")
            nc.vector.tensor_tensor(out=a1[:], in0=t_[:], in1=pcst["cosg"][:], op=AL.mult)
            a2 = pc.tile([128, BT], F32, tag="a2")
            nc.vector.tensor_tensor(out=a2[:], in0=sq[:], in1=pcst["sing"][:], op=AL.mult)
            cosm = pc.tile([128, BT], F32, tag="cosm")
            nc.vector.tensor_tensor(out=cosm[:], in0=a1[:], in1=a2[:], op=AL.subtract)

            ml2 = pc.tile([128, BT], F32, tag="ml2")
            nc.vector.tensor_tensor(
                out=ml2[:], in0=t_[:], in1=pcst["thr_lo"][:], op=AL.is_gt
            )
            mlow = pc.tile([128, BT], F32, tag="mlow")
            nc.vector.tensor_tensor(out=mlow[:], in0=pcst["ml1"][:], in1=ml2[:], op=AL.mult)
            mh2 = pc.tile([128, BT], F32, tag="mh2")
            nc.vector.tensor_tensor(
                out=mh2[:], in0=t_[:], in1=pcst["nthr"][:], op=AL.is_lt
            )
            mhigh = pc.tile([128, BT], F32, tag="mhigh")
            nc.vector.tensor_tensor(out=mhigh[:], in0=pcst["mh1"][:], in1=mh2[:], op=AL.mult)

            mlow_i = pc.tile([128, BT], I32, tag="mlow_i")
            nc.vector.tensor_copy(mlow_i[:], mlow[:])
            mhigh_i = pc.tile([128, BT], I32, tag="mhigh_i")
            nc.vector.tensor_copy(mhigh_i[:], mhigh[:])
            nc.vector.select(cosm[:], mlow_i[:], pcst["c_lo"][:], cosm[:])
            nc.vector.select(cosm[:], mhigh_i[:], pcst["c_hi"][:], cosm[:])

            val = pc.tile([128, BT], F32, tag="val")
            nc.vector.tensor_tensor(
                out=val[:], in0=cosm[:], in1=gadd_sb[:], op=AL.subtract
            )
            nc.vector.tensor_scalar(val[:], val[:], S, None, op0=AL.mult)
            nc.scalar.dma_start(out=fix_h[:, :], in_=val[:])

        # ---- emission ----
        load_small()
        load_embT()
        ksbs = {0: load_round(0, split_k=True), 1: load_round(1),
                2: load_round(2)}
        load_rowcols()
        phase_a()
        pcst = phase_c_pre()

        loaded = 3
        osbs = None
        for ri in range(len(rounds)):
            while loaded < len(rounds) and loaded <= ri + 3:
                ksbs[loaded] = load_round(loaded)
                loaded += 1
            if ri % 2 == 0:
                osbs = [
                    op_.tile([128, 2048], BF16, tag=f"o{b}", name=f"o{b}")
                    for b in range(BT)
                ]
            main_round(ri, ksbs[ri], osbs)
            if ri < BT:
                phase_c_dot(pcst, ri)
            elif ri == BT:
                phase_c_vals(pcst)

    cst_cm.__exit__(None, None, None)


def _build():
    nc = bacc.Bacc(
        "TRN2", target_bir_lowering=False, debug=False, num_devices=NCORES
    )
    embT_h = nc.dram_tensor("embT", [128, KT * B], BF16, kind="ExternalInput")
    kern_h = nc.dram_tensor("kern", [128, KT * CS], BF16, kind="ExternalInput")
    labf_h = nc.dram_tensor("labf", [128, BT], F32, kind="ExternalInput")
    nrm_h = nc.dram_tensor("nrm", [128, BT], F32, kind="ExternalInput")
    labrow_h = nc.dram_tensor("labrow", [1, B], F32, kind="ExternalInput")
    embrow_h = nc.dram_tensor("embrow", [128, BT * EMB], BF16, kind="ExternalInput")
    kncol_h = nc.dram_tensor("kncol", [128, BT * EMB], BF16, kind="ExternalInput")
    fix_h = nc.dram_tensor("fix", [128, BT], F32, kind="ExternalOutput")
    out_hs = [
        nc.dram_tensor(f"out{b}", [128 * CS, 1], BF16, kind="ExternalOutput")
        for b in range(BT)
    ]
    with tile.TileContext(nc) as tc:
        _emit(nc, tc, embT_h, kern_h, labf_h, nrm_h, labrow_h,
              embrow_h, kncol_h, fix_h, out_hs)
    nc.compile()
    return nc


_NC = None


def _get_nc():
    global _NC
    if _NC is None:
        _NC = _build()
    return _NC


def _prep_inputs(embbedings, norms, label, kernel):
    import ml_dtypes

    bf16 = ml_dtypes.bfloat16
    emb = np.asarray(embbedings, dtype=np.float32)
    nrm = np.asarray(norms, dtype=np.float32).reshape(B)
    lab = np.asarray(label).astype(np.int64).reshape(B)
    kern = np.asarray(kernel, dtype=np.float32)

    # fold column normalization and the S scale into the bf16 kernel
    colnorm = np.sqrt((kern * kern).sum(axis=0))
    knS = np.zeros((EMB, CS * NCORES), dtype=np.float32)
    knS[:, :C] = kern * (S / colnorm)
    knS16 = knS.astype(bf16)

    e16 = emb.astype(bf16)
    embT_arr = np.ascontiguousarray(
        e16.T.reshape(KT, 128, B).transpose(1, 0, 2).reshape(128, KT * B)
    )
    embrow_arr = np.ascontiguousarray(
        e16.reshape(BT, 128, EMB).transpose(1, 0, 2).reshape(128, BT * EMB)
    )
    kncg = np.ascontiguousarray(knS16[:, lab].T)  # [B, EMB]
    kncol_arr = np.ascontiguousarray(
        kncg.reshape(BT, 128, EMB).transpose(1, 0, 2).reshape(128, BT * EMB)
    )
    nrm_arr = np.ascontiguousarray(nrm.reshape(BT, 128).T)

    rounds = _rounds()
    in_maps = []
    for c in range(NCORES):
        kc4 = knS16[:, c * CS : (c + 1) * CS].reshape(KT, 128, CS)
        kern_arr = np.concatenate(
            [
                kc4[:, :, c0 : c0 + W].transpose(1, 0, 2).reshape(128, KT * W)
                for (c0, W) in rounds
            ],
            axis=1,
        )
        la = (lab - c * CS).astype(np.int32)
        labf_arr = np.ascontiguousarray(la.reshape(BT, 128).T).astype(np.float32)
        labrow_arr = la.astype(np.float32).reshape(1, B)
        in_maps.append(
            {
                "embT": embT_arr,
                "kern": np.ascontiguousarray(kern_arr),
                "labf": labf_arr,
                "nrm": nrm_arr,
                "labrow": labrow_arr,
                "embrow": embrow_arr,
                "kncol": kncol_arr,
            }
        )
    return in_maps, lab


def _run(in_maps, **kwargs):
    nc = _get_nc()
    return run_bass_kernel_spmd(nc, in_maps, core_ids=list(range(NCORES)), **kwargs)


def _assemble(res, lab):
    parts = []
    for c in range(NCORES):
        rows = [res.results[c][f"out{b}"].reshape(128, CS) for b in range(BT)]
        parts.append(np.concatenate(rows, axis=0))
    out = np.concatenate(parts, axis=1)[:, :C].astype(np.float32)
    # place the device-computed margin fix-up values at (i, label_i)
    fix = np.asarray(res.results[0]["fix"], dtype=np.float32)  # [128, BT]
    out[np.arange(B), lab] = fix.T.reshape(B)
    return out


def kernel(embbedings, norms, label, kernel):
    in_maps, lab = _prep_inputs(embbedings, norms, label, kernel)
    res = _run(in_maps)
    return _assemble(res, lab)


# revision 34
# speedup vs baseline: 1.2199x; 1.1119x over previous
"""CWCFace head (nn_CWCFace_11201274708637) — Trainium2 Bass kernel.

Math (reference):
    kn = kernel / ||kernel||_col
    cos = clip(emb @ kn, -1+eps, 1-eps)              # [B, C]
    ms  = margin_scaler(norms, label)                # [B, 1] per-sample stats
    th  = arccos(cos); th_m = clip(th + onehot*(-M*ms), eps, pi-eps)
    out = (cos(th_m) - onehot*(M + M*ms)) * S

The onehot terms touch exactly ONE column per row, so the full [B, C]
tensor only needs  out = clip(S*cos)  plus a B-element fix-up at
(i, label_i).  cos(th+g) for those B elements uses the identity
cos(th+g) = t*cos(g) - sqrt(1-t^2)*sin(g); the theta-clip branches are
threshold comparisons — no arccos.

Device kernel = one big bf16 matmul + clamp + scatter fix-up:
  - the column normalization and the S scale are folded into the bf16
    kernel upload (host-side input prep), so psum is S*cos directly
  - epilogue is a single DVE tensor_scalar clamp PSUM(f32) -> SBUF bf16,
    output DMA is bf16 (half the write traffic of f32)
  - fix-up values come from per-sample dot products against the label's
    kernel column (host gathers the columns; device does the math), so
    no gather-after-store serial tail — just 4 indirect scatters that
    wait on the stores of their B-tile
  - margin stats (segment count/sum/sumsq) via BxB label-equality
    matmul, as before

Sharding: classes column-split over 8 cores, CS=8848 per core
(8*8848 = 70784 >= 70722).  Kernel blocks are uploaded pre-swizzled to
[128, KT*W] so every block load is one long contiguous DMA per
partition; blocks are prefetched 2 ahead to keep the PE gap-free (and
at the max p-state).
"""

import sys

for _p in (
    "/root/.axon_site",
    "/root/.axon_site/_ro/trn_rl_repo",
    "/root/.axon_site/_ro/pypackages",
    "/opt/trn_rl_repo",
):
    if _p not in sys.path:
        sys.path.append(_p)

import math

import numpy as np

import concourse.bass as bass
import concourse.mybir as mybir
import concourse.tile as tile
from concourse import bacc
from concourse.bass import IndirectOffsetOnAxis
from concourse.bass_utils import run_bass_kernel_spmd

B = 512
EMB = 512
C = 70722
NCORES = 8
CS = 8848  # per-core classes (padded);  8 * 8848 = 70784 >= 70722
S = 64.0
MARG = 0.4
H = 0.333
EPS = 1e-3

F32 = mybir.dt.float32
BF16 = mybir.dt.bfloat16
I32 = mybir.dt.int32
AL = mybir.AluOpType
AF = mybir.ActivationFunctionType

KT = EMB // 128          # 4 K-tiles
BT = B // 128            # 4 B-tiles
COS_EPS = float(math.cos(EPS))
PI_2 = math.pi / 2.0
CLAMP = S * (1.0 - EPS)


def _rounds():
    """Compute rounds: (c0, W).  W=1024 fills half of PSUM per B-tile so
    two rounds can be in flight; the short tail round drains fast."""
    out = []
    c0 = 0
    while c0 < CS:
        w = min(1024, CS - c0)
        out.append((c0, w))
        c0 += w
    return out


def _emit(nc, tc, embT_h, kern_h, labf_h, nrm_h, labA_h,
          embrow_h, kncol_h, fix_h, out_hs):
    out2ds = [
        oh[:, :].rearrange("(p c) o -> p (c o)", c=CS) for oh in out_hs
    ]  # [128, CS] each

    cst_cm = tc.tile_pool(name="cst", bufs=1)
    cst = cst_cm.__enter__()

    embT_sb = cst.tile([128, KT, B], BF16, tag="embT")      # [p, k, b]
    embrow_sb = cst.tile([128, BT, EMB], BF16, tag="embrow")
    kncol_sb = cst.tile([128, BT, EMB], BF16, tag="kncol")
    labf_sb = cst.tile([128, BT], F32, tag="labf")
    nrm_sb = cst.tile([128, BT], F32, tag="nrm")
    labA_sb = cst.tile([128, B], F32, tag="labA")
    g_sb = cst.tile([128, BT], F32, tag="g")        # -M * ms
    gadd_sb = cst.tile([128, BT], F32, tag="gadd")  # M + M * ms
    v_sb = cst.tile([128, BT], F32, tag="v")        # safe norms

    def load_embT():
        nc.sync.dma_start(
            out=embT_sb[:],
            in_=embT_h[:, :].rearrange("p (k b) -> p k b", k=KT),
        )

    def load_small():
        # phase-A inputs first: it can start the moment these land
        nc.sync.dma_start(out=labf_sb[:], in_=labf_h[:, :])
        nc.sync.dma_start(out=nrm_sb[:], in_=nrm_h[:, :])
        nc.sync.dma_start(out=labA_sb[:], in_=labA_h[:, :])

    def load_rowcols():
        # only needed by phase_c_vals (~25us in) — keep off the start path
        nc.sync.dma_start(out=embrow_sb[:], in_=embrow_h[:, :].rearrange(
            "p (t e) -> p t e", t=BT))
        nc.sync.dma_start(out=kncol_sb[:], in_=kncol_h[:, :].rearrange(
            "p (t e) -> p t e", t=BT))

    rounds = _rounds()
    kernR = kern_h  # [128, KT*CS], round r at columns KT*c0 .. KT*(c0+W)

    with (
        tc.tile_pool(name="pa", bufs=2) as pa,
        tc.tile_pool(name="kp", bufs=4) as kp,
        tc.tile_pool(name="op", bufs=2) as op_,
        tc.tile_pool(name="ps", bufs=4, space="PSUM") as ps,
        tc.tile_pool(name="pc", bufs=1) as pc,
    ):
        def load_round(ri, split_k=False):
            c0, W = rounds[ri]
            ksb = kp.tile([128, KT, W], BF16, tag="ks")
            src = kernR[:, KT * c0 : KT * (c0 + W)].rearrange(
                "p (k w) -> p k w", k=KT
            )
            if split_k:
                for k in range(KT):
                    nc.sync.dma_start(out=ksb[:, k, :], in_=src[:, k, :])
            else:
                nc.sync.dma_start(out=ksb[:], in_=src)
            return ksb

        def main_round(ri, ksb, osbs):
            """One round: 4 B-tiles x [128, W] psum, one clamp per B-tile.
            Clamped bf16 goes into the round-pair staging tile osbs[b]; the
            [128, 2048] store is issued (from the otherwise-idle Scalar
            queue) when the pair completes."""
            c0, W = rounds[ri]
            half = (ri % 2) * 1024
            for b in range(BT):
                ps_out = ps.tile([128, W], F32, space="PSUM", tag="po")
                for k in range(KT):
                    for j in range(0, W, 512):
                        wj = min(512, W - j)
                        nc.tensor.matmul(
                            ps_out[:, j : j + wj],
                            embT_sb[:, k, b * 128 : (b + 1) * 128],
                            ksb[:, k, j : j + wj],
                            start=(k == 0),
                            stop=(k == KT - 1),
                        )
                nc.vector.tensor_scalar(
                    osbs[b][:, half : half + W], ps_out[:],
                    -CLAMP, CLAMP, op0=AL.max, op1=AL.min,
                )
                if ri == len(rounds) - 1 or ri % 2 == 1:
                    p0 = c0 - half
                    eng = nc.scalar if b % 2 == 0 else nc.sync
                    eng.dma_start(
                        out=out2ds[b][:, p0 : p0 + half + W],
                        in_=osbs[b][:, : half + W],
                    )

        def phase_a():
            """Margin-scaler segment stats -> g_sb, gadd_sb.  eq/w are bf16
            (exact for 0/1 indicators and small counts) so the stat matmuls
            run at full PE rate instead of fp32 mode."""
            nc.vector.tensor_scalar(
                v_sb[:], nrm_sb[:], 0.001, 100.0, op0=AL.max, op1=AL.min
            )
            w_sb = pa.tile([128, 3 * BT], BF16, tag="w")
            nc.vector.memset(w_sb[:], 1.0)
            for b in range(BT):
                nc.vector.tensor_copy(
                    w_sb[:, 3 * b + 1 : 3 * b + 2], v_sb[:, b : b + 1]
                )
                nc.vector.tensor_tensor(
                    out=w_sb[:, 3 * b + 2 : 3 * b + 3],
                    in0=v_sb[:, b : b + 1],
                    in1=v_sb[:, b : b + 1],
                    op=AL.mult,
                )

            st_all = pa.tile([128, 3 * BT], F32, tag="st_all")
            for a in range(BT):
                ps_st = ps.tile([128, 3], F32, space="PSUM", tag="po")
                for b in range(BT):
                    eq = pa.tile([128, 128], BF16, tag="eq")
                    nc.vector.tensor_tensor(
                        out=eq[:],
                        in0=labf_sb[:, b : b + 1].to_broadcast([128, 128]),
                        in1=labA_sb[:, a * 128 : (a + 1) * 128],
                        op=AL.is_equal,
                    )
                    nc.tensor.matmul(
                        ps_st[:],
                        eq[:],
                        w_sb[:, 3 * b : 3 * b + 3],
                        start=(b == 0),
                        stop=(b == BT - 1),
                    )
                nc.vector.tensor_copy(st_all[:, 3 * a : 3 * a + 3], ps_st[:])

            stv = st_all[:].rearrange("p (a c) -> p a c", c=3)
            n_ = stv[:, :, 0]
            sm = stv[:, :, 1]
            sq2 = stv[:, :, 2]

            t0 = pa.tile([128, 8 * BT], F32, tag="t0")
            tv = t0[:].rearrange("p (i a) -> p i a", a=BT)
            rn = tv[:, 0, :]
            nc.vector.reciprocal(rn, n_)
            mean = tv[:, 1, :]
            nc.vector.tensor_tensor(out=mean, in0=sm, in1=rn, op=AL.mult)
            m2 = tv[:, 2, :]
            nc.vector.tensor_tensor(out=m2, in0=mean, in1=mean, op=AL.mult)
            nm2 = tv[:, 3, :]
            nc.vector.tensor_tensor(out=nm2, in0=n_, in1=m2, op=AL.mult)
            num = tv[:, 4, :]
            nc.vector.tensor_tensor(out=num, in0=sq2, in1=nm2, op=AL.subtract)
            den = tv[:, 5, :]
            nc.vector.tensor_scalar(den, n_, -1.0, 1.0, op0=AL.add, op1=AL.max)
            rden = tv[:, 6, :]
            nc.vector.reciprocal(rden, den)
            var = tv[:, 7, :]
            nc.vector.tensor_tensor(out=var, in0=num, in1=rden, op=AL.mult)
            nc.vector.tensor_scalar(var, var, 1e-30, None, op0=AL.max)

            t1 = pa.tile([128, 8 * BT], F32, tag="t1")
            uv = t1[:].rearrange("p (i a) -> p i a", a=BT)
            ars = uv[:, 0, :]
            nc.scalar.activation(ars, var, AF.Abs_reciprocal_sqrt)
            std = uv[:, 1, :]
            nc.vector.tensor_tensor(out=std, in0=var, in1=ars, op=AL.mult)
            stdp = uv[:, 2, :]
            nc.vector.tensor_scalar(stdp, std, EPS, None, op0=AL.add)
            rstd = uv[:, 3, :]
            nc.vector.reciprocal(rstd, stdp)
            mask = uv[:, 4, :]
            nc.vector.tensor_scalar(mask, n_, 2.0, None, op0=AL.is_gt)
            mask_i = pa.tile([128, BT], I32, tag="mask_i")
            nc.vector.tensor_copy(mask_i[:], mask)
            c05 = uv[:, 5, :]
            nc.vector.memset(c05, 0.05)
            invd = uv[:, 6, :]
            nc.vector.select(invd, mask_i[:], rstd, c05)
            dv = uv[:, 7, :]
            nc.vector.tensor_tensor(out=dv, in0=v_sb[:], in1=mean, op=AL.subtract)
            res = tv[:, 0, :]
            nc.vector.tensor_tensor(out=res, in0=dv, in1=invd, op=AL.mult)
            ms = tv[:, 1, :]
            nc.vector.tensor_scalar(ms, res, H, 1.0, op0=AL.mult, op1=AL.min)
            nc.vector.tensor_scalar(ms, ms, -1.0, None, op0=AL.max)
            nc.vector.tensor_scalar(g_sb[:], ms, -MARG, None, op0=AL.mult)
            nc.vector.tensor_scalar(
                gadd_sb[:], ms, MARG, MARG, op0=AL.mult, op1=AL.add
            )

        def phase_c_pre():
            """Angle thresholds (needs g_sb)."""
            pcst = {}
            cpi2 = pc.tile([128, 1], F32, tag="cpi2")
            nc.vector.memset(cpi2[:], PI_2)
            cpie = pc.tile([128, 1], F32, tag="cpie")
            nc.vector.memset(cpie[:], PI_2 + EPS)

            cosg = pc.tile([128, BT], F32, tag="cosg")
            sing = pc.tile([128, BT], F32, tag="sing")
            thr_lo = pc.tile([128, BT], F32, tag="thr_lo")
            thr_hi = pc.tile([128, BT], F32, tag="thr_hi")
            for b in range(BT):
                gb = g_sb[:, b : b + 1]
                nc.scalar.activation(cosg[:, b : b + 1], gb, AF.Sin, bias=cpi2[:])
                nc.scalar.activation(sing[:, b : b + 1], gb, AF.Sin)
                nc.scalar.activation(
                    thr_lo[:, b : b + 1], gb, AF.Sin, bias=cpie[:], scale=-1.0
                )
                nc.scalar.activation(
                    thr_hi[:, b : b + 1], gb, AF.Sin, bias=cpie[:], scale=1.0
                )
            nthr = pc.tile([128, BT], F32, tag="nthr")
            nc.vector.tensor_scalar(nthr[:], thr_hi[:], -1.0, None, op0=AL.mult)
            ml1 = pc.tile([128, BT], F32, tag="ml1")
            nc.vector.tensor_scalar(ml1[:], g_sb[:], EPS, None, op0=AL.is_lt)
            mh1 = pc.tile([128, BT], F32, tag="mh1")
            nc.vector.tensor_scalar(mh1[:], g_sb[:], -EPS, None, op0=AL.is_gt)
            c_lo = pc.tile([128, BT], F32, tag="c_lo")
            nc.vector.memset(c_lo[:], COS_EPS)
            c_hi = pc.tile([128, BT], F32, tag="c_hi")
            nc.vector.memset(c_hi[:], -COS_EPS)
            pcst.update(
                cosg=cosg, sing=sing, thr_lo=thr_lo, nthr=nthr,
                ml1=ml1, mh1=mh1, c_lo=c_lo, c_hi=c_hi,
            )
            return pcst

        def phase_c_dot(pcst, b):
            """One per-B-tile dot product vs the label's kernel column;
            spread across rounds so the DVE never falls behind the PE."""
            if "traw" not in pcst:
                pcst["traw"] = pc.tile([128, BT], F32, tag="traw", name="traw")
                pcst["scr"] = pc.tile([128, BT, EMB], F32, tag="scr", name="scr")
            nc.vector.scalar_tensor_tensor(
                out=pcst["scr"][:, b, :],
                in0=embrow_sb[:, b, :],
                scalar=1.0,
                in1=kncol_sb[:, b, :],
                op0=AL.mult,
                op1=AL.mult,
                accum_out=pcst["traw"][:, b : b + 1],
            )

        def phase_c_vals(pcst):
            """Fix-up values from direct dot products (no HBM gather)."""
            traw = pcst["traw"]
            t_ = pc.tile([128, BT], F32, tag="t_")
            nc.vector.tensor_scalar(
                t_[:], traw[:], 1.0 / S, 1.0 - EPS, op0=AL.mult, op1=AL.min
            )
            nc.vector.tensor_scalar(t_[:], t_[:], -1.0 + EPS, None, op0=AL.max)

            t2 = pc.tile([128, BT], F32, tag="t2")
            nc.vector.tensor_tensor(out=t2[:], in0=t_[:], in1=t_[:], op=AL.mult)
            om = pc.tile([128, BT], F32, tag="om")
            nc.vector.tensor_scalar(om[:], t2[:], -1.0, 1.0, op0=AL.mult, op1=AL.add)
            omr = pc.tile([128, BT], F32, tag="omr")
            nc.scalar.activation(omr[:], om[:], AF.Abs_reciprocal_sqrt)
            sq = pc.tile([128, BT], F32, tag="sq")
            nc.vector.tensor_tensor(out=sq[:], in0=om[:], in1=omr[:], op=AL.mult)

            a1 = pc.tile([128, BT], F32, tag="a1")
            nc.vector.tensor_tensor(out=a1[:], in0=t_[:], in1=pcst["cosg"][:], op=AL.mult)
            a2 = pc.tile([128, BT], F32, tag="a2")
            nc.vector.tensor_tensor(out=a2[:], in0=sq[:], in1=pcst["sing"][:], op=AL.mult)
            cosm = pc.tile([128, BT], F32, tag="cosm")
            nc.vector.tensor_tensor(out=cosm[:], in0=a1[:], in1=a2[:], op=AL.subtract)

            ml2 = pc.tile([128, BT], F32, tag="ml2")
            nc.vector.tensor_tensor(
                out=ml2[:], in0=t_[:], in1=pcst["thr_lo"][:], op=AL.is_gt
            )
            mlow = pc.tile([128, BT], F32, tag="mlow")
            nc.vector.tensor_tensor(out=mlow[:], in0=pcst["ml1"][:], in1=ml2[:], op=AL.mult)
            mh2 = pc.tile([128, BT], F32, tag="mh2")
            nc.vector.tensor_tensor(
                out=mh2[:], in0=t_[:], in1=pcst["nthr"][:], op=AL.is_lt
            )
            mhigh = pc.tile([128, BT], F32, tag="mhigh")
            nc.vector.tensor_tensor(out=mhigh[:], in0=pcst["mh1"][:], in1=mh2[:], op=AL.mult)

            mlow_i = pc.tile([128, BT], I32, tag="mlow_i")
            nc.vector.tensor_copy(mlow_i[:], mlow[:])
            mhigh_i = pc.tile([128, BT], I32, tag="mhigh_i")
            nc.vector.tensor_copy(mhigh_i[:], mhigh[:])
            nc.vector.select(cosm[:], mlow_i[:], pcst["c_lo"][:], cosm[:])
            nc.vector.select(cosm[:], mhigh_i[:], pcst["c_hi"][:], cosm[:])

            val = pc.tile([128, BT], F32, tag="val")
            nc.vector.tensor_tensor(
                out=val[:], in0=cosm[:], in1=gadd_sb[:], op=AL.subtract
            )
            nc.vector.tensor_scalar(val[:], val[:], S, None, op0=AL.mult)
            nc.scalar.dma_start(out=fix_h[:, :], in_=val[:])

        # ---- emission ----
        load_small()
        load_embT()
        ksbs = {0: load_round(0, split_k=True), 1: load_round(1),
                2: load_round(2)}
        load_rowcols()
        phase_a()
        pcst = phase_c_pre()

        loaded = 3
        osbs = None
        for ri in range(len(rounds)):
            while loaded < len(rounds) and loaded <= ri + 3:
                ksbs[loaded] = load_round(loaded)
                loaded += 1
            if ri % 2 == 0:
                osbs = [
                    op_.tile([128, 2048], BF16, tag=f"o{b}", name=f"o{b}")
                    for b in range(BT)
                ]
            main_round(ri, ksbs[ri], osbs)
            if ri < BT:
                phase_c_dot(pcst, ri)
            elif ri == BT:
                phase_c_vals(pcst)

    cst_cm.__exit__(None, None, None)


def _build():
    nc = bacc.Bacc(
        "TRN2", target_bir_lowering=False, debug=False, num_devices=NCORES
    )
    embT_h = nc.dram_tensor("embT", [128, KT * B], BF16, kind="ExternalInput")
    kern_h = nc.dram_tensor("kern", [128, KT * CS], BF16, kind="ExternalInput")
    labf_h = nc.dram_tensor("labf", [128, BT], F32, kind="ExternalInput")
    nrm_h = nc.dram_tensor("nrm", [128, BT], F32, kind="ExternalInput")
    labA_h = nc.dram_tensor("labA", [128, B], F32, kind="ExternalInput")
    embrow_h = nc.dram_tensor("embrow", [128, BT * EMB], BF16, kind="ExternalInput")
    kncol_h = nc.dram_tensor("kncol", [128, BT * EMB], BF16, kind="ExternalInput")
    fix_h = nc.dram_tensor("fix", [128, BT], F32, kind="ExternalOutput")
    out_hs = [
        nc.dram_tensor(f"out{b}", [128 * CS, 1], BF16, kind="ExternalOutput")
        for b in range(BT)
    ]
    with tile.TileContext(nc) as tc:
        _emit(nc, tc, embT_h, kern_h, labf_h, nrm_h, labA_h,
              embrow_h, kncol_h, fix_h, out_hs)
    nc.compile()
    return nc


_NC = None


def _get_nc():
    global _NC
    if _NC is None:
        _NC = _build()
    return _NC


def _prep_inputs(embbedings, norms, label, kernel):
    import ml_dtypes

    bf16 = ml_dtypes.bfloat16
    emb = np.asarray(embbedings, dtype=np.float32)
    nrm = np.asarray(norms, dtype=np.float32).reshape(B)
    lab = np.asarray(label).astype(np.int64).reshape(B)
    kern = np.asarray(kernel, dtype=np.float32)

    # fold column normalization and the S scale into the bf16 kernel
    colnorm = np.sqrt((kern * kern).sum(axis=0))
    knS = np.zeros((EMB, CS * NCORES), dtype=np.float32)
    knS[:, :C] = kern * (S / colnorm)
    knS16 = knS.astype(bf16)

    e16 = emb.astype(bf16)
    embT_arr = np.ascontiguousarray(
        e16.T.reshape(KT, 128, B).transpose(1, 0, 2).reshape(128, KT * B)
    )
    embrow_arr = np.ascontiguousarray(
        e16.reshape(BT, 128, EMB).transpose(1, 0, 2).reshape(128, BT * EMB)
    )
    kncg = np.ascontiguousarray(knS16[:, lab].T)  # [B, EMB]
    kncol_arr = np.ascontiguousarray(
        kncg.reshape(BT, 128, EMB).transpose(1, 0, 2).reshape(128, BT * EMB)
    )
    nrm_arr = np.ascontiguousarray(nrm.reshape(BT, 128).T)

    rounds = _rounds()
    in_maps = []
    for c in range(NCORES):
        kc4 = knS16[:, c * CS : (c + 1) * CS].reshape(KT, 128, CS)
        kern_arr = np.concatenate(
            [
                kc4[:, :, c0 : c0 + W].transpose(1, 0, 2).reshape(128, KT * W)
                for (c0, W) in rounds
            ],
            axis=1,
        )
        la = (lab - c * CS).astype(np.int32)
        labf_arr = np.ascontiguousarray(la.reshape(BT, 128).T).astype(np.float32)
        labA_arr = np.ascontiguousarray(
            np.broadcast_to(la.astype(np.float32), (128, B))
        )
        in_maps.append(
            {
                "embT": embT_arr,
                "kern": np.ascontiguousarray(kern_arr),
                "labf": labf_arr,
                "nrm": nrm_arr,
                "labA": labA_arr,
                "embrow": embrow_arr,
                "kncol": kncol_arr,
            }
        )
    return in_maps, lab


def _run(in_maps, **kwargs):
    nc = _get_nc()
    return run_bass_kernel_spmd(nc, in_maps, core_ids=list(range(NCORES)), **kwargs)


def _assemble(res, lab):
    parts = []
    for c in range(NCORES):
        rows = [res.results[c][f"out{b}"].reshape(128, CS) for b in range(BT)]
        parts.append(np.concatenate(rows, axis=0))
    out = np.concatenate(parts, axis=1)[:, :C].astype(np.float32)
    # place the device-computed margin fix-up values at (i, label_i)
    fix = np.asarray(res.results[0]["fix"], dtype=np.float32)  # [128, BT]
    out[np.arange(B), lab] = fix.T.reshape(B)
    return out


def kernel(embbedings, norms, label, kernel):
    in_maps, lab = _prep_inputs(embbedings, norms, label, kernel)
    res = _run(in_maps)
    return _assemble(res, lab)
